# revision 7
# baseline (speedup 1.0000x reference)
"""Trainium2 Bass kernel for a NetVLAD-style VQ codebook module (v3).

reference semantics (B=16, N=2048, D=1024, K=64):
    x = l2norm(grids, axis=D)
    logits = x @ W.T + b            # [B, N, K]
    a = softmax(logits, axis=K)
    p = einsum('bnk,bnd->bkd', a, x) - centroids * a.sum(n)
    out = l2norm(p, axis=D)

Sharding: data-parallel over batch B across 8 cores (2 batches/core).

v3 changes over v2 (35.4us -> target ~30us):
  - DMA split across TWO engine queues (SP + Pool/SWDGE) in strict
    alternation so per-instruction HWDGE/DGE overheads hide under the other
    queue's transfer; shared DMA device stays saturated.  Identity constants
    come in via DMA (host-side numpy) instead of Pool-engine affine_select.
  - b==0 fast path (always true for this module): esum via DVE tensor_reduce
    directly on e = exp(r'z'); the e~/ebb STT pass and et_sb disappear.
  - v (rank-1 rsc correction) folded into the mm2 DR pair loop (64 DR
    matmuls instead of 128 1-col matmuls), gated per gn chunk.
  - batch-1 tail: gn1 lands last in (h1a, h0, h1b-small) order; finalize
    splits final scaling across DVE (h0) and gpsimd (h1) in parallel; out
    DMAs issued from idle queues (DVE: out0, ACT: out1h0, SP: out1h1).
  - rn via quake-rsqrt (iters=1) on DVE; rnb read from PSUM (no copy).
"""

import os
import sys

sys.path.insert(0, "/opt/trn_rl_repo")

import numpy as np

import concourse.bacc as bacc
import concourse.bass as bass
import concourse.mybir as mybir
import concourse.tile as tile

B, N, D, K = 16, 2048, 1024, 64
N_CORES = 8
LB = B // N_CORES
NT = N // 128              # 16 n-tiles
DC = 8                     # 8 d-chunks
GW = 4                     # gram wave size (tiles per PSUM wave)
GRP = 8                    # softmax group size (tiles)
NG = NT // GRP

F32 = mybir.dt.float32
F16 = mybir.dt.float16
F8 = mybir.dt.float8e4
I32 = mybir.dt.int32
ALU = mybir.AluOpType
ACTF = mybir.ActivationFunctionType
PM = mybir.MatmulPerfMode
AXL = mybir.AxisListType

RSQRT_MAGIC = 0x5F3759DF

_CACHE = {}


def _emit_rsqrt(nc, pool, y, x, scratch_shape, iters=2, tag="rsq", eng=None):
    """y = x**-0.5 via quake-seed + Newton (DVE by default)."""
    e = eng if eng is not None else nc.vector
    t = pool.tile(scratch_shape, F32, tag=tag + "_t")
    u = pool.tile(scratch_shape, F32, tag=tag + "_u")
    e.tensor_scalar(
        out=t.bitcast(I32), in0=x.bitcast(I32),
        scalar1=1, scalar2=None, op0=ALU.arith_shift_right)
    e.tensor_scalar(
        out=y.bitcast(I32), in0=t.bitcast(I32),
        scalar1=-1, scalar2=None, op0=ALU.bitwise_xor)
    e.tensor_scalar(
        out=y.bitcast(I32), in0=y.bitcast(I32),
        scalar1=RSQRT_MAGIC + 1, scalar2=None, op0=ALU.add)
    for _ in range(iters):
        e.tensor_mul(t, y, y)
        e.tensor_mul(u, t, x)
        e.tensor_scalar(
            out=u, in0=u, scalar1=-0.5, scalar2=1.5, op0=ALU.mult, op1=ALU.add)
        e.tensor_mul(y, y, u)


def _gen_nc(b_zero=True):
    nc = bacc.Bacc(None, target_bir_lowering=False)

    gt_d = nc.dram_tensor("gt", [LB, DC, 128, N], F8, kind="ExternalInput")
    gn_d = nc.dram_tensor("gn", [LB, 128, NT, D], F8, kind="ExternalInput")
    wt_d = nc.dram_tensor("wt", [DC, 128, K], F8, kind="ExternalInput")
    bias_d = nc.dram_tensor("bias", [1, K], F32, kind="ExternalInput")
    cent_d = nc.dram_tensor("cent", [K, D], F16, kind="ExternalInput")
    idf_d = nc.dram_tensor("idf", [128, 128], F32, kind="ExternalInput")
    id16_d = nc.dram_tensor("id16", [128, 128], F16, kind="ExternalInput")
    id64_d = nc.dram_tensor("id64", [K, K], F16, kind="ExternalInput")
    out_d = nc.dram_tensor("out", [LB, 128, DC, K], F16, kind="ExternalOutput")

    from contextlib import ExitStack

    with tile.TileContext(nc) as tc, ExitStack() as ctx:
        singles = ctx.enter_context(tc.tile_pool(name="singles", bufs=1))
        gpool = ctx.enter_context(tc.tile_pool(name="gpool", bufs=2))
        work = ctx.enter_context(tc.tile_pool(name="work", bufs=2))
        scr = ctx.enter_context(tc.tile_pool(name="scr", bufs=2))
        ps_gram = ctx.enter_context(tc.tile_pool(name="ps_gram", bufs=2, space="PSUM"))
        ps_z = ctx.enter_context(tc.tile_pool(name="ps_z", bufs=1, space="PSUM"))
        ps_p1 = ctx.enter_context(tc.tile_pool(name="ps_p1", bufs=2, space="PSUM"))
        ps_small = ctx.enter_context(tc.tile_pool(name="ps_small", bufs=2, space="PSUM"))

        # ---- SBUF destination tiles ----
        wt_sb = singles.tile([128, DC, K], F8)
        bias_sb = singles.tile([1, K], F32)
        cent_sb = singles.tile([K, D], F16)
        identF = singles.tile([128, 128], F32)
        ident16 = singles.tile([128, 128], F16)
        ident64 = singles.tile([K, K], F16)

        gt_sbs = [gpool.tile([128, DC, N], F8, name=f"gt_sb{lb}", tag="gt")
                  for lb in range(LB)]
        gn_sbs = [gpool.tile([128, NT, D], F8, name=f"gn_sb{lb}", tag="gn")
                  for lb in range(LB)]

        # ---- input DMA emission ----
        # Strict SP/Pool alternation on the shared DMA device:
        #   wt(SP), gt0h1(Pool), gt0h0(SP), gt1h1(Pool), gt1h0(SP),
        #   gn0h1(Pool), gn0h0(SP), gn1h1a(Pool), gn1h0(SP), gn1h1b(Pool)
        # Each queue's next prep happens under the other queue's transfer.
        # h1 chunks land first within each tensor -> compute processes
        # halves in [1, 0] order.
        def gt_dma(eng, lb, h):
            ns = slice(h * (N // 2), (h + 1) * (N // 2))
            eng.dma_start(
                out=gt_sbs[lb][:, :, ns],
                in_=gt_d[lb, :, :, ns].rearrange("c p n -> p c n"))

        def gn_dma(eng, lb, t0, t1):
            eng.dma_start(
                out=gn_sbs[lb][:, t0:t1, :],
                in_=gn_d[lb, :, t0:t1, :])

        nc.sync.dma_start(out=wt_sb, in_=wt_d.rearrange("c p k -> p c k"))
        nc.gpsimd.dma_start(out=identF, in_=idf_d[:])
        gt_dma(nc.sync, 0, 0)
        nc.gpsimd.dma_start(out=ident16, in_=id16_d[:])
        gt_dma(nc.gpsimd, 0, 1)
        gt_dma(nc.sync, 1, 0)
        gt_dma(nc.gpsimd, 1, 1)
        gn_dma(nc.sync, 0, 0, NT // 2)
        gn_dma(nc.gpsimd, 0, NT // 2, NT)
        gn_dma(nc.sync, 1, 0, NT // 2)           # gn1 h0: tiles 0-7
        gn_dma(nc.gpsimd, 1, NT // 2, NT - 2)    # gn1 h1a: tiles 8-13
        gn_dma(nc.gpsimd, 1, NT - 2, NT)         # gn1 h1b: tiles 14-15
        nc.scalar.dma_start(out=bias_sb, in_=bias_d[:])
        nc.scalar.dma_start(out=cent_sb, in_=cent_d[:])
        nc.scalar.dma_start(out=ident64, in_=id64_d[:])

        ones128 = singles.tile([128, 1], F16)
        nc.vector.memset(ones128, 1.0)
        ones64p = singles.tile([K, 128], F16)
        nc.vector.memset(ones64p, 1.0)

        # exp(b) broadcast (general-b path only)
        if not b_zero:
            ones1p = singles.tile([1, 128], F16)
            nc.vector.memset(ones1p, 1.0)
            eb16 = singles.tile([1, K], F16)
            nc.scalar.activation(out=eb16, in_=bias_sb, func=ACTF.Exp)
            ebb_ps = ps_small.tile([128, 512], F32, tag="small", name="ebb_small")
            nc.tensor.matmul(ebb_ps[:, 0:K], ones1p[:, :], eb16[:, :],
                             start=True, stop=True)
            ebb16 = singles.tile([128, K], F16)
            nc.vector.tensor_copy(out=ebb16, in_=ebb_ps[:, 0:K])

        # ---- per-batch state ----
        st = []
        for lb in range(LB):
            keys = [
                ("s_h0", [128, NT // 2], F32),
                ("s_h1", [128, NT // 2], F32),
                ("r_h0", [128, NT // 2], F32),
                ("r_h1", [128, NT // 2], F32),
                ("e_sb", [128, NT, K], F16),
                ("dl_sb", [128, NT, K], F8),
                ("esum", [128, NT], F32),
                ("recip", [128, NT], F32),
                ("rtmp", [128, NT], F32),
                ("rsc16", [128, NT], F16),
                ("nrecipS", [128, NT], F16),
                ("D16", [K, K], F16),
                ("v16", [128, DC], F16),
            ]
            if not b_zero:
                keys.append(("et_sb", [128, NT, K], F16))
            st.append({
                k: work.tile(shp, dt, tag=k, name=f"{k}{lb}")
                for k, shp, dt in keys
            })

        def phase_A(lb):
            """grams + diag extract + rsqrt + mm1, halves in [1, 0] order."""
            S = st[lb]
            gt_sb = gt_sbs[lb]
            gtp = gt_sb.rearrange("p (i two) n -> p i two n", two=2)
            wtp = wt_sb.rearrange("p (i two) k -> p i two k", two=2)
            S["zhs"] = [None, None]
            for half in (1, 0):
                zps = ps_z.tile([128, NT // 2, K], F32, tag=f"z{half}",
                                name=f"z{lb}_{half}")
                S["zhs"][half] = zps
                s_h = S[f"s_h{half}"]
                r_h = S[f"r_h{half}"]
                waves = [half * (NT // GW // 2) + w for w in range(NT // GW // 2)]
                for w in waves:
                    gps = ps_gram.tile([128, GW, 128], F32, tag="gram",
                                       name=f"gram{lb}_{w}")
                    for ti in range(GW):
                        t = w * GW + ti
                        lt = gtp[:, :, :, t * 128:(t + 1) * 128]
                        for i in range(DC // 2):
                            nc.tensor.matmul(
                                gps[:, ti, :], lt[:, i], lt[:, i],
                                start=(i == 0), stop=(i == DC // 2 - 1),
                                perf_mode=PM.DoubleRow)
                    for ti in range(GW):
                        t = w * GW + ti
                        th = t - half * (NT // 2)
                        sc_t = scr.tile([128, 128], F32, tag="xscr")
                        nc.vector.scalar_tensor_tensor(
                            out=sc_t, in0=gps[:, ti, :], scalar=1024.0,
                            in1=identF, op0=ALU.mult, op1=ALU.mult,
                            accum_out=s_h[:, th:th + 1])
                _emit_rsqrt(nc, work, r_h, s_h, [128, NT // 2])
                for t in range(half * (NT // 2), (half + 1) * (NT // 2)):
                    for i in range(DC // 2):
                        nc.tensor.matmul(
                            zps[:, t - half * (NT // 2), :],
                            gtp[:, i, :, t * 128:(t + 1) * 128],
                            wtp[:, i], start=(i == 0), stop=(i == DC // 2 - 1),
                            perf_mode=PM.DoubleRow)

        def phase_B(lb, g):
            """softmax group g: exp, esum, group scalars, delta, pac."""
            S = st[lb]
            if g == 1:  # first-processed group allocates the small PSUM tile
                small = ps_small.tile([128, 512], F32, tag="small",
                                      name=f"small{lb}")
                S["small"] = small
                S["pac"] = small[0:K, 0:1]
            src = S["e_sb"] if b_zero else S["et_sb"]
            for t in range(g * GRP, (g + 1) * GRP):
                half = t // (NT // 2)
                th = t % (NT // 2)
                zt = S["zhs"][half][:, th, :]
                r_h = S[f"r_h{half}"]
                nc.scalar.activation(
                    out=S["e_sb"][:, t, :], in_=zt,
                    func=ACTF.Exp, scale=r_h[:, th:th + 1])
                if b_zero:
                    nc.vector.tensor_reduce(
                        out=S["esum"][:, t:t + 1], in_=S["e_sb"][:, t, :],
                        axis=AXL.X, op=ALU.add)
                else:
                    nc.vector.scalar_tensor_tensor(
                        out=S["et_sb"][:, t, :], in0=S["e_sb"][:, t, :],
                        scalar=1.0, in1=ebb16, op0=ALU.mult, op1=ALU.mult,
                        accum_out=S["esum"][:, t:t + 1])
            gs = slice(g * GRP, (g + 1) * GRP)
            nc.vector.reciprocal(out=S["recip"][:, gs], in_=S["esum"][:, gs])
            half = (g * GRP) // (NT // 2)
            hs = slice(g * GRP - half * (NT // 2),
                       (g + 1) * GRP - half * (NT // 2))
            nc.vector.tensor_mul(S["rtmp"][:, gs], S["recip"][:, gs],
                                 S[f"r_h{half}"][:, hs])
            nc.vector.tensor_scalar_mul(S["rsc16"][:, gs], S["rtmp"][:, gs],
                                        2.0 ** 20)
            nc.vector.tensor_scalar_mul(S["nrecipS"][:, gs],
                                        S["recip"][:, gs], -(2.0 ** 9))
            first_t = NT // 2 if g == 1 else None
            for t in range(g * GRP, (g + 1) * GRP):
                nc.vector.scalar_tensor_tensor(
                    out=S["dl_sb"][:, t, :], in0=src[:, t, :],
                    scalar=-1.0,
                    in1=S["rsc16"][:, t:t + 1].broadcast_to((128, K)),
                    op0=ALU.add, op1=ALU.mult)
                nc.tensor.matmul(
                    S["pac"], src[:, t, :], S["nrecipS"][:, t:t + 1],
                    start=(t == NT // 2), stop=(t == NT // 2 - 1))
            if g == 0:
                # pac group closed; D16 = ident64 * diag-scalar(pac)
                nc.vector.tensor_scalar(
                    out=S["D16"], in0=ident64, scalar1=S["pac"], scalar2=None,
                    op0=ALU.mult)

        def phase_M(lb, prs, first, last):
            """mm2 + v accumulation for gn pair-rows `prs` (DR pairs)."""
            S = st[lb]
            if first:
                p1 = ps_p1.tile([128, DC, K], F32, tag="p1", name=f"p1_{lb}")
                S["p1"] = p1
            p1 = S["p1"]
            v_ps = S["small"][:, 8:8 + DC]
            dlp = S["dl_sb"].rearrange("p (pr two) k -> p pr two k", two=2)
            gnp = gn_sbs[lb].rearrange("p (pr two) d -> p pr two d", two=2)
            gn_sb = gn_sbs[lb]
            for j, pr in enumerate(prs):
                for c in range(DC):
                    nc.tensor.matmul(
                        p1[:, c, :],
                        gnp[:, pr, :, c * 128:(c + 1) * 128],
                        dlp[:, pr],
                        start=(first and j == 0 and c == 0),
                        stop=False,
                        perf_mode=PM.DoubleRow)
                for k2, t in enumerate((2 * pr, 2 * pr + 1)):
                    for c in range(DC):
                        nc.tensor.matmul(
                            v_ps[:, c:c + 1],
                            gn_sb[:, t, c * 128:(c + 1) * 128],
                            S["rsc16"][:, t:t + 1],
                            start=(first and j == 0 and k2 == 0 and c == 0),
                            stop=(last and j == len(prs) - 1 and k2 == 1
                                  and c == DC - 1))

        def phase_C(lb):
            """close p1: + v broadcast + cent correction."""
            S = st[lb]
            p1 = S["p1"]
            v_ps = S["small"][:, 8:8 + DC]
            v16 = S["v16"]
            nc.vector.tensor_copy(out=v16, in_=v_ps)
            for c in range(DC):
                nc.tensor.matmul(
                    p1[:, c, :], ident16[:, :],
                    v16[:, c:c + 1].broadcast_to((128, K)),
                    start=False, stop=False)
            for c in range(DC):
                nc.tensor.matmul(
                    p1[:, c, :], cent_sb[:, c * 128:(c + 1) * 128], S["D16"],
                    start=False, stop=(c == DC - 1))

        def phase_D(lb):
            """finalize: qq, n2, rn, rnb, scale, out DMA (per half)."""
            S = st[lb]
            small = S["small"]
            p1 = S["p1"]
            n2_ps = small[0:K, 16 + DC:16 + DC + 1]
            rnb_ps = small[:, 128:128 + K]
            qq16 = work.tile([128, DC, K], F16, tag="qq")
            hc = DC // 2
            for h in range(2):
                cs = slice(h * hc, (h + 1) * hc)
                nc.scalar.activation(out=qq16[:, cs, :], in_=p1[:, cs, :],
                                     func=ACTF.Square, scale=2.0 ** -10)
                for c in range(h * hc, (h + 1) * hc):
                    nc.tensor.matmul(
                        n2_ps, qq16[:, c, :], ones128[:, 0:1],
                        start=(c == 0), stop=(c == DC - 1))
            n2_sb = work.tile([K, 1], F32, tag="n2sb")
            nc.vector.tensor_copy(out=n2_sb, in_=n2_ps)
            rn_sb = work.tile([K, 1], F32, tag="rn")
            _emit_rsqrt(nc, work, rn_sb, n2_sb, [K, 1], iters=1, tag="rsq2")
            rnD = work.tile([K, K], F16, tag="rnD")
            nc.vector.tensor_scalar(
                out=rnD, in0=ident64, scalar1=rn_sb, scalar2=None, op0=ALU.mult)
            nc.tensor.matmul(rnb_ps, ones64p[:, :], rnD[:, :],
                             start=True, stop=True)
            rnb16 = work.tile([128, K], F16, tag="rnb16")
            nc.vector.tensor_copy(out=rnb16, in_=rnb_ps)
            out_sb = work.tile([128, DC, K], F16, tag="out_sb")
            for h in range(2):
                cs = slice(h * hc, (h + 1) * hc)
                eng = nc.vector
                eng.scalar_tensor_tensor(
                    out=out_sb[:, cs, :], in0=p1[:, cs, :], scalar=2.0 ** -10,
                    in1=rnb16.unsqueeze(1).broadcast_to((128, hc, K)),
                    op0=ALU.mult, op1=ALU.mult)
                if lb == 1:
                    deng = nc.scalar if h == 0 else nc.sync
                    deng.dma_start(out=out_d[lb, :, cs], in_=out_sb[:, cs, :])
            if lb == 0:
                nc.gpsimd.dma_start(out=out_d[lb, :, :], in_=out_sb[:, :, :])

        # ---- schedule ----
        phase_A(0)
        phase_B(0, 1)
        phase_B(0, 0)
        phase_A(1)
        phase_M(0, [4, 5, 6, 7], first=True, last=False)   # gn0 h1
        phase_B(1, 1)
        phase_M(0, [0, 1, 2, 3], first=False, last=True)   # gn0 h0
        phase_C(0)
        phase_B(1, 0)
        phase_D(0)
        phase_M(1, [4, 5, 6], first=True, last=False)      # gn1 h1a (t8-13)
        phase_M(1, [0, 1, 2, 3], first=False, last=False)  # gn1 h0 (t0-7)
        phase_M(1, [7], first=False, last=True)            # gn1 h1b (t14-15)
        phase_C(1)
        phase_D(1)

    nc.compile()
    return nc


def _get_nc(b_zero=True):
    key = ("nc", b_zero)
    if key not in _CACHE:
        _CACHE[key] = _gen_nc(b_zero)
    return _CACHE[key]


def _prep_core_inputs(grids, W, b, centroids):
    """Host-side prep: fp8/fp16 casts + per-core sharded layouts."""
    import ml_dtypes

    f8 = ml_dtypes.float8_e4m3fn
    gn = np.ascontiguousarray(
        grids.astype(f8).reshape(B, NT, 128, D).transpose(0, 2, 1, 3))
    gt = np.ascontiguousarray(
        grids.transpose(0, 2, 1)).astype(f8).reshape(B, DC, 128, N)
    wt = np.ascontiguousarray((W * 32.0).T.astype(f8)).reshape(DC, 128, K)
    bias = b.astype(np.float32).reshape(1, K)
    cent = (centroids * 64.0).astype(np.float16).reshape(K, D)
    idf = np.eye(128, dtype=np.float32)
    id16 = np.eye(128, dtype=np.float16)
    id64 = np.eye(K, dtype=np.float16)

    in_maps = []
    for c in range(N_CORES):
        sl = slice(c * LB, (c + 1) * LB)
        in_maps.append(
            {
                "gt": np.ascontiguousarray(gt[sl]),
                "gn": np.ascontiguousarray(gn[sl]),
                "wt": wt,
                "bias": bias,
                "cent": cent,
                "idf": idf,
                "id16": id16,
                "id64": id64,
            }
        )
    return in_maps


def kernel(idx, grids, W, b, centroids):
    from concourse.bass_utils import run_bass_kernel_spmd

    b_np = np.asarray(b, dtype=np.float32)
    nc = _get_nc(b_zero=not np.any(b_np))
    in_maps = _prep_core_inputs(
        np.asarray(grids, dtype=np.float32),
        np.asarray(W, dtype=np.float32),
        b_np,
        np.asarray(centroids, dtype=np.float32),
    )
    res = run_bass_kernel_spmd(nc, in_maps, core_ids=list(range(N_CORES)))
    outs = []
    for c in range(N_CORES):
        o = res.results[c]["out"]                  # [LB, 128, DC, K] f16
        o = np.transpose(o.astype(np.float32), (0, 3, 2, 1))  # [LB, K, DC, 128]
        outs.append(o.reshape(LB, K, D))
    return np.concatenate(outs, axis=0)


# revision 66
# speedup vs baseline: 1.3685x; 1.3685x over previous
"""Trainium2 Bass kernel for a NetVLAD-style VQ codebook module.

reference semantics (B=16, N=2048, D=1024, K=64):
    x = l2norm(grids, axis=D)
    logits = x @ W.T + b            # [B, N, K]
    a = softmax(logits, axis=K)
    p = einsum('bnk,bnd->bkd', a, x) - centroids * a.sum(n)
    out = l2norm(p, axis=D)

Sharding: data-parallel over batch B across 8 cores (2 batches/core).

Design (35.4us -> 26.4us vs the v2 baseline):
  - G is fed twice in fp8 (gt d-major for grams/mm1, gn n-major for mm2);
    the 8MB input stream is issued from all three DMA-capable queues
    (SP/ACT/Pool) which execute concurrently, landing by ~11us; early gt
    halves are quartered so the first gram wave starts at ~3us.  Identity
    constants arrive via DMA so Pool is free to issue from t=0.
  - Row norms s=||g||^2 via PE fp8-DoubleRow Gram tiles; diagonals
    extracted with per-tile DVE masked-STT accumulates; r = quake-rsqrt
    (fused 2-op seed, 1 Newton iter).
  - Softmax: per-tile ACT exp with per-partition scale ptr r (grouped
    DVE-prescale + one big exp for the laggard half); esum as one grouped
    DVE tensor_reduce per half; delta = (e-1)*rsc in fp8 via grouped DVE
    STTs (gpsimd cannot run TensorScalarPtr on HW).
  - mm2 in [d,k] orientation, fp8 DoubleRow pairs; rank-1 rsc correction
    (v) accumulated by separate 1-col matmuls gated only on rsc+gn, all
    sharing one PSUM bank with pac via a single 128-partition zero-region
    group; centroid correction via cent^T @ diag(-asum) closes p1.
  - Finalize: ACT Square + p116 staging, ones-matmul row norms, quake
    rn, PE rank-1 broadcast; final scaling split DVE/gpsimd-TensorTensor;
    out DMAs issued from ACT/SP/Pool queue tails.
  - Emission order hand-scheduled so the four (batch, half) softmax
    pipelines flow in DMA-arrival order and no engine queue head-of-line
    blocks another batch's critical chain.
"""

import os
import sys

sys.path.insert(0, "/opt/trn_rl_repo")

import numpy as np

import concourse.bacc as bacc
import concourse.bass as bass
import concourse.mybir as mybir
import concourse.tile as tile

B, N, D, K = 16, 2048, 1024, 64
N_CORES = 8
LB = B // N_CORES
NT = N // 128              # 16 n-tiles
DC = 8                     # 8 d-chunks
GW = 4                     # gram wave size (tiles per PSUM wave)
HT = NT // 2               # tiles per half

F32 = mybir.dt.float32
F16 = mybir.dt.float16
F8 = mybir.dt.float8e4
I32 = mybir.dt.int32
ALU = mybir.AluOpType
ACTF = mybir.ActivationFunctionType
PM = mybir.MatmulPerfMode
AXL = mybir.AxisListType

RSQRT_MAGIC = 0x5F3759DF

_CACHE = {}


def _emit_rsqrt(nc, pool, y, x, scratch_shape, iters=2, tag="rsq", eng=None):
    """y = x**-0.5 via quake-seed + Newton (DVE by default)."""
    e = eng if eng is not None else nc.vector
    t = pool.tile(scratch_shape, F32, tag=tag + "_t")
    u = pool.tile(scratch_shape, F32, tag=tag + "_u")
    # y = MAGIC - (x>>1) == ((x>>1) ^ -1) + (MAGIC+1), fused into 2 ops
    e.tensor_scalar(
        out=t.bitcast(I32), in0=x.bitcast(I32),
        scalar1=1, scalar2=-1, op0=ALU.arith_shift_right, op1=ALU.bitwise_xor)
    e.tensor_scalar(
        out=y.bitcast(I32), in0=t.bitcast(I32),
        scalar1=RSQRT_MAGIC + 1, scalar2=None, op0=ALU.add)
    for _ in range(iters):
        e.tensor_mul(t, y, y)
        e.tensor_mul(u, t, x)
        e.tensor_scalar(
            out=u, in0=u, scalar1=-0.5, scalar2=1.5, op0=ALU.mult, op1=ALU.add)
        e.tensor_mul(y, y, u)


def _gen_nc(b_zero=True):
    nc = bacc.Bacc(None, target_bir_lowering=False)

    gt_d = nc.dram_tensor("gt", [LB, DC, 128, N], F8, kind="ExternalInput")
    gn_d = nc.dram_tensor("gn", [LB, 128, NT, D], F8, kind="ExternalInput")
    wt_d = nc.dram_tensor("wt", [DC, 128, K], F8, kind="ExternalInput")
    bias_d = nc.dram_tensor("bias", [1, K], F32, kind="ExternalInput")
    cent_d = nc.dram_tensor("cent", [K, D], F16, kind="ExternalInput")
    idf_d = nc.dram_tensor("idf", [128, 128], F32, kind="ExternalInput")
    id16_d = nc.dram_tensor("id16", [128, 128], F16, kind="ExternalInput")
    id64_d = nc.dram_tensor("id64", [K, K], F16, kind="ExternalInput")
    out_d = nc.dram_tensor("out", [LB, 128, DC, K], F16, kind="ExternalOutput")

    from contextlib import ExitStack

    with tile.TileContext(nc) as tc, ExitStack() as ctx:
        singles = ctx.enter_context(tc.tile_pool(name="singles", bufs=1))
        gpool = ctx.enter_context(tc.tile_pool(name="gpool", bufs=2))
        work = ctx.enter_context(tc.tile_pool(name="work", bufs=2))
        scr = ctx.enter_context(tc.tile_pool(name="scr", bufs=2))
        ps_gram = ctx.enter_context(tc.tile_pool(name="ps_gram", bufs=2, space="PSUM"))
        ps_z = ctx.enter_context(tc.tile_pool(name="ps_z", bufs=1, space="PSUM"))
        ps_p1 = ctx.enter_context(tc.tile_pool(name="ps_p1", bufs=2, space="PSUM"))
        ps_small = ctx.enter_context(tc.tile_pool(name="ps_small", bufs=2, space="PSUM"))

        # ---- SBUF destination tiles ----
        wt_sb = singles.tile([128, DC, K], F8)
        bias_sb = singles.tile([1, K], F32)
        cent_sb = singles.tile([K, D], F16)
        identF = singles.tile([128, 128], F32)
        ident16 = singles.tile([128, 128], F16)
        ident64 = singles.tile([K, K], F16)

        gt_sbs = [gpool.tile([128, DC, N], F8, name=f"gt_sb{lb}", tag="gt")
                  for lb in range(LB)]
        gn_sbs = [gpool.tile([128, NT, D], F8, name=f"gn_sb{lb}", tag="gn")
                  for lb in range(LB)]

        # ---- input DMA emission (3 concurrent queues) ----
        def gt_dma(eng, lb, t0, t1):
            ns = slice(t0 * 128, t1 * 128)
            eng.dma_start(
                out=gt_sbs[lb][:, :, ns],
                in_=gt_d[lb, :, :, ns].rearrange("c p n -> p c n"))

        def gn_dma(eng, lb, t0, t1):
            eng.dma_start(
                out=gn_sbs[lb][:, t0:t1, :],
                in_=gn_d[lb, :, t0:t1, :])

        # SP queue: gt0h0 quartered for earliest gram start, then gt1h1, gn1
        gt_dma(nc.sync, 0, 0, 4)
        gt_dma(nc.sync, 0, 4, 8)
        gt_dma(nc.sync, 1, 8, 12)
        gt_dma(nc.sync, 1, 12, 16)
        gn_dma(nc.sync, 1, 0, NT // 2)           # gn1 h0 (t0-7)
        gn_dma(nc.sync, 1, NT - 2, NT)           # gn1 h1b (t14-15)
        # ACT queue (frees by ~7.5us for exp work)
        nc.scalar.dma_start(out=wt_sb, in_=wt_d.rearrange("c p k -> p c k"))
        gt_dma(nc.scalar, 0, 8, 12)
        gt_dma(nc.scalar, 0, 12, 16)
        gn_dma(nc.sync, 0, 0, NT // 2)           # gn0 h0 (SP tail; needed late)
        # Pool queue (frees by ~9us for the delta offload)
        nc.gpsimd.dma_start(out=identF, in_=idf_d[:])
        gt_dma(nc.gpsimd, 1, 0, 4)
        gt_dma(nc.gpsimd, 1, 4, 8)
        gn_dma(nc.gpsimd, 0, NT // 2, NT)        # gn0 h1
        gn_dma(nc.gpsimd, 1, NT // 2, NT - 2)    # gn1 h1a (t8-13)
        nc.gpsimd.dma_start(out=ident16, in_=id16_d[:])
        nc.gpsimd.dma_start(out=ident64, in_=id64_d[:])
        nc.gpsimd.dma_start(out=cent_sb, in_=cent_d[:])
        nc.gpsimd.dma_start(out=bias_sb, in_=bias_d[:])

        ones128 = singles.tile([128, 1], F16)
        nc.vector.memset(ones128, 1.0)
        ones64p = singles.tile([K, 128], F16)
        nc.vector.memset(ones64p, 1.0)


        if not b_zero:
            ones1p = singles.tile([1, 128], F16)
            nc.vector.memset(ones1p, 1.0)
            eb16 = singles.tile([1, K], F16)
            nc.scalar.activation(out=eb16, in_=bias_sb, func=ACTF.Exp)
            ebb_ps = ps_small.tile([128, 512], F32, tag="small", name="ebb_small")
            nc.tensor.matmul(ebb_ps[:, 0:K], ones1p[:, :], eb16[:, :],
                             start=True, stop=True)
            ebb16 = singles.tile([128, K], F16)
            nc.vector.tensor_copy(out=ebb16, in_=ebb_ps[:, 0:K])

        # ---- per-batch state ----
        st = []
        for lb in range(LB):
            keys = [
                ("s_h0", [128, HT], F32),
                ("s_h1", [128, HT], F32),
                ("r_h0", [128, HT], F32),
                ("r_h1", [128, HT], F32),
                ("e_sb", [128, NT, K], F16),
                ("dl_sb", [128, NT, K], F8),
                ("esum", [128, NT], F32),
                ("recip", [128, NT], F32),
                ("rsc16", [128, NT], F16),
                ("nrecipS", [128, NT], F16),
                ("D16", [K, K], F16),
                ("v16", [128, DC], F16),
            ]
            if not b_zero:
                keys.append(("et_sb", [128, NT, K], F16))
            st.append({
                k: work.tile(shp, dt, tag=k, name=f"{k}{lb}")
                for k, shp, dt in keys
            })

        def phase_Ag(lb, half):
            """grams + diag extract + rsqrt for one half of gt."""
            S = st[lb]
            gtp = gt_sbs[lb].rearrange("p (i two) n -> p i two n", two=2)
            s_h = S[f"s_h{half}"]
            r_h = S[f"r_h{half}"]
            for w in range(HT // GW):
                wt_ = half * (HT // GW) + w
                gps = ps_gram.tile([128, GW, 128], F32, tag="gram",
                                   name=f"gram{lb}_{wt_}")
                for ti in range(GW):
                    t = wt_ * GW + ti
                    lt = gtp[:, :, :, t * 128:(t + 1) * 128]
                    for i in range(DC // 2):
                        nc.tensor.matmul(
                            gps[:, ti, :], lt[:, i], lt[:, i],
                            start=(i == 0), stop=(i == DC // 2 - 1),
                            perf_mode=PM.DoubleRow)
                for ti in range(GW):
                    t = wt_ * GW + ti
                    th = t - half * HT
                    sc_t = scr.tile([128, 128], F32, tag="xscr")
                    nc.vector.scalar_tensor_tensor(
                        out=sc_t, in0=gps[:, ti, :], scalar=1024.0,
                        in1=identF, op0=ALU.mult, op1=ALU.mult,
                        accum_out=s_h[:, th:th + 1])
            _emit_rsqrt(nc, work, r_h, s_h, [128, HT], iters=1)

        z_count = [0]

        def phase_Az(lb, half):
            """mm1 for one half -> zps[half]."""
            S = st[lb]
            gtp = gt_sbs[lb].rearrange("p (i two) n -> p i two n", two=2)
            wtp = wt_sb.rearrange("p (i two) k -> p i two k", two=2)
            # alternate between two PSUM banks in emission order
            ztag = f"z{z_count[0] % 2}"
            z_count[0] += 1
            zps = ps_z.tile([128, HT, K], F32, tag=ztag,
                            name=f"z{lb}_{half}")
            S.setdefault("zhs", [None, None])[half] = zps
            for t in range(half * HT, (half + 1) * HT):
                for i in range(DC // 2):
                    nc.tensor.matmul(
                        zps[:, t - half * HT, :],
                        gtp[:, i, :, t * 128:(t + 1) * 128],
                        wtp[:, i], start=(i == 0), stop=(i == DC // 2 - 1),
                        perf_mode=PM.DoubleRow)

        def phase_Be(lb, half, grouped=False, accum=False):
            """exp of one half: per-tile with scale ptr (optionally with the
            ACT row-sum accumulator computing esum), or DVE-prescale + one
            grouped ACT op (shorter chain; used for the last half)."""
            S = st[lb]
            zps = S["zhs"][half]
            r_h = S[f"r_h{half}"]
            gs = slice(half * HT, (half + 1) * HT)
            if grouped:
                zs16 = work.tile([128, HT, K], F16, tag="zs16")
                nc.vector.scalar_tensor_tensor(
                    out=zs16, in0=zps[:, :, :], scalar=1.0,
                    in1=r_h.unsqueeze(2).broadcast_to((128, HT, K)),
                    op0=ALU.mult, op1=ALU.mult)
                nc.scalar.activation(
                    out=S["e_sb"][:, gs, :], in_=zs16, func=ACTF.Exp)
            else:
                for t in range(half * HT, (half + 1) * HT):
                    th = t - half * HT
                    acc = S["esum"][:, t:t + 1] if (accum and b_zero) else None
                    nc.scalar.activation(
                        out=S["e_sb"][:, t, :], in_=zps[:, th, :],
                        func=ACTF.Exp, scale=r_h[:, th:th + 1],
                        accum_out=acc)

        def phase_Bs(lb, half, esum_done=False, pooled=False):
            """esum (group reduce; optionally avg-pool on gpsimd with the
            1/64 folded into the group scalars) + group scalars."""
            S = st[lb]
            gs = slice(half * HT, (half + 1) * HT)
            rsc_scale, nrec_scale = 2.0 ** 20, -(2.0 ** 9)
            if b_zero:
                if pooled:
                    # esum/64 via gpsimd avg-pool (InstPool is in the
                    # standard gpsimd library); constants absorb the 64x.
                    # opt=False keeps the [128,HT,K] dims so the pool
                    # window stays K (the optimizer would merge them).
                    from concourse import ap_utils
                    in_pap = nc.gpsimd.lower_ap(S["e_sb"][:, gs, :],
                                                opt=False)
                    nd = len(in_pap.ap)
                    in_pap.ap = mybir.VecI64Pair(ap_utils.expand_dims_ap(
                        in_pap.ap, [i for i in range(1, 6 - nd)]))
                    nc.gpsimd.add_instruction(mybir.InstPool(
                        name=nc.get_next_instruction_name(),
                        func=mybir.PoolFunctionType.avg,
                        ins=[in_pap],
                        outs=[nc.gpsimd.lower_ap(S["esum"][:, gs])]))
                    rsc_scale, nrec_scale = 2.0 ** 14, -(2.0 ** 3)
                elif not esum_done:
                    nc.vector.tensor_reduce(
                        out=S["esum"][:, gs], in_=S["e_sb"][:, gs, :],
                        axis=AXL.X, op=ALU.add)
            else:
                for t in range(half * HT, (half + 1) * HT):
                    nc.vector.scalar_tensor_tensor(
                        out=S["et_sb"][:, t, :], in0=S["e_sb"][:, t, :],
                        scalar=1.0, in1=ebb16, op0=ALU.mult, op1=ALU.mult,
                        accum_out=S["esum"][:, t:t + 1])
            nc.vector.reciprocal(out=S["recip"][:, gs], in_=S["esum"][:, gs])
            # rsc = recip * r * scale (one fused STT)
            nc.vector.scalar_tensor_tensor(
                out=S["rsc16"][:, gs], in0=S["recip"][:, gs],
                scalar=rsc_scale, in1=S[f"r_h{half}"],
                op0=ALU.mult, op1=ALU.mult)
            nc.vector.tensor_scalar_mul(S["nrecipS"][:, gs],
                                        S["recip"][:, gs], nrec_scale)

        def phase_Bd(lb, half, pac_first, pac_last, eng=None):
            """delta (group STT, gpsimd by default) + pac matmuls."""
            S = st[lb]
            src = S["e_sb"] if b_zero else S["et_sb"]
            gs = slice(half * HT, (half + 1) * HT)
            rb = S["rsc16"][:, gs].unsqueeze(2).broadcast_to((128, HT, K))
            # gpsimd can only run TensorTensor on HW, so deltas run on DVE
            # (grouped STT) or ACT (per-tile Identity with scale/bias ptrs)
            if eng is nc.scalar:
                nrsc = work.tile([128, HT], F32, tag="nrsc")
                nc.scalar.activation(out=nrsc, in_=S["rsc32"][:, gs],
                                     func=ACTF.Identity, scale=-1.0)
                for t in range(half * HT, (half + 1) * HT):
                    th = t - half * HT
                    nc.scalar.activation(
                        out=S["dl_sb"][:, t, :], in_=src[:, t, :],
                        func=ACTF.Identity,
                        scale=S["rsc32"][:, t:t + 1],
                        bias=nrsc[:, th:th + 1])
            else:
                deng = eng if eng is not None else nc.vector
                deng.scalar_tensor_tensor(
                    out=S["dl_sb"][:, gs, :], in0=src[:, gs, :],
                    scalar=-1.0, in1=rb, op0=ALU.add, op1=ALU.mult)
            ts = list(range(half * HT, (half + 1) * HT))
            for t in ts:
                nc.tensor.matmul(
                    S["pac"], src[:, t, :], S["nrecipS"][:, t:t + 1],
                    start=False, stop=False, skip_group_check=True)

        def alloc_small(lb):
            S = st[lb]
            small = ps_small.tile([128, 512], F32, tag="small",
                                  name=f"small{lb}")
            S["small"] = small
            S["pac"] = small[0:K, 0:1]

        def phase_D16(lb):
            S = st[lb]
            nc.vector.tensor_scalar(
                out=S["D16"], in0=ident64, scalar1=S["pac"], scalar2=None,
                op0=ALU.mult)

        def phase_Mv(lb, prs, first, last):
            """v accumulation for gn pair-rows (needs only rsc + gn).

            The v group's start (a 128-partition op) zero-marks the whole
            PSUM bank; the 64-partition pac matmuls ride along with
            skip_group_check and get lazily zeroed on first write."""
            S = st[lb]
            v_ps = S["small"][:, 8:8 + DC]
            gn_sb = gn_sbs[lb]
            for j, pr in enumerate(prs):
                for k2, t in enumerate((2 * pr, 2 * pr + 1)):
                    for c in range(DC):
                        nc.tensor.matmul(
                            v_ps[:, c:c + 1],
                            gn_sb[:, t, c * 128:(c + 1) * 128],
                            S["rsc16"][:, t:t + 1],
                            start=(first and j == 0 and k2 == 0 and c == 0),
                            stop=(last and j == len(prs) - 1 and k2 == 1
                                  and c == DC - 1))

        def phase_M(lb, prs, first, last):
            """mm2 (fp8 DR) for gn pair-rows `prs`."""
            S = st[lb]
            if first:
                p1 = ps_p1.tile([128, DC, K], F32, tag="p1", name=f"p1_{lb}")
                S["p1"] = p1
            p1 = S["p1"]
            dlp = S["dl_sb"].rearrange("p (pr two) k -> p pr two k", two=2)
            gnp = gn_sbs[lb].rearrange("p (pr two) d -> p pr two d", two=2)
            for j, pr in enumerate(prs):
                for c in range(DC):
                    nc.tensor.matmul(
                        p1[:, c, :],
                        gnp[:, pr, :, c * 128:(c + 1) * 128],
                        dlp[:, pr],
                        start=(first and j == 0 and c == 0),
                        stop=False,
                        perf_mode=PM.DoubleRow)

        def phase_Cv(lb):
            S = st[lb]
            nc.vector.tensor_copy(out=S["v16"], in_=S["small"][:, 8:8 + DC])

        def phase_C(lb):
            """close p1: + v broadcast + cent correction."""
            S = st[lb]
            p1 = S["p1"]
            for c in range(DC):
                nc.tensor.matmul(
                    p1[:, c, :], ident16[:, :],
                    S["v16"][:, c:c + 1].broadcast_to((128, K)),
                    start=False, stop=False)
            for c in range(DC):
                nc.tensor.matmul(
                    p1[:, c, :], cent_sb[:, c * 128:(c + 1) * 128], S["D16"],
                    start=False, stop=(c == DC - 1))

        def phase_D(lb):
            """finalize: qq (ACT||DVE halves), n2, rn, rnb, scale, out DMA."""
            S = st[lb]
            small = S["small"]
            p1 = S["p1"]
            n2_ps = small[0:K, 16 + DC:16 + DC + 1]
            rnb_ps = small[:, 128:128 + K]
            qq16 = work.tile([128, DC, K], F16, tag="qq")
            p116 = work.tile([128, DC, K], F16, tag="p116")
            hc = DC // 2
            h0 = slice(0, hc)
            h1 = slice(hc, DC)
            # ACT squares the whole tile, then stages p116 while PE sums n2
            nc.scalar.activation(out=qq16, in_=p1,
                                 func=ACTF.Square, scale=2.0 ** -10)
            nc.scalar.activation(out=p116, in_=p1,
                                 func=ACTF.Identity, scale=2.0 ** -10)
            for c in range(DC):
                nc.tensor.matmul(
                    n2_ps, qq16[:, c, :], ones128[:, 0:1],
                    start=(c == 0), stop=(c == DC - 1))
            n2_sb = work.tile([K, 1], F32, tag="n2sb")
            nc.vector.tensor_copy(out=n2_sb, in_=n2_ps)
            rn_sb = work.tile([K, 1], F32, tag="rn")
            rne = nc.vector
            _emit_rsqrt(nc, work, rn_sb, n2_sb, [K, 1], iters=1, tag="rsq2",
                        eng=rne)
            rnD = work.tile([K, K], F16, tag="rnD")
            rne.tensor_scalar(
                out=rnD, in0=ident64, scalar1=rn_sb, scalar2=None, op0=ALU.mult)
            nc.tensor.matmul(rnb_ps, ones64p[:, :], rnD[:, :],
                             start=True, stop=True)
            rnb16 = work.tile([128, K], F16, tag="rnb16")
            nc.vector.tensor_copy(out=rnb16, in_=rnb_ps)
            out_sb = work.tile([128, DC, K], F16, tag="out_sb")
            S["out_sb"] = out_sb
            if lb == 0:
                # tail batch: final scaling halves in parallel DVE || gpsimd
                # (TensorTensor — the only elementwise op gpsimd runs on HW)
                nc.vector.tensor_tensor(
                    out=out_sb[:, h0, :], in0=p116[:, h0, :],
                    in1=rnb16.unsqueeze(1).broadcast_to((128, hc, K)),
                    op=ALU.mult)
                nc.gpsimd.tensor_tensor(
                    out=out_sb[:, h1, :], in0=p116[:, h1, :],
                    in1=rnb16.unsqueeze(1).broadcast_to((128, hc, K)),
                    op=ALU.mult)
            else:
                # keep DVE free for the tail batch's rn chain
                nc.gpsimd.tensor_tensor(
                    out=out_sb, in0=p116,
                    in1=rnb16.unsqueeze(1).broadcast_to((128, DC, K)),
                    op=ALU.mult)

        def phase_Dout(lb):
            S = st[lb]
            out_sb = S["out_sb"]
            hc = DC // 2
            if lb == 0:
                # batch0 finishes last -> fast split out path
                nc.scalar.dma_start(out=out_d[lb, :, 0:hc],
                                    in_=out_sb[:, 0:hc, :])
                nc.sync.dma_start(out=out_d[lb, :, hc:DC],
                                  in_=out_sb[:, hc:DC, :])
            else:
                nc.gpsimd.dma_start(out=out_d[lb, :, :], in_=out_sb[:, :, :])

        # ---- schedule (emission order == per-engine queue order) ----
        # Half pipelines flow in order (0,0), (1,1), (1,0), (0,1) matching
        # DMA arrival; diag/rsqrt prioritized on DVE so no half's softmax
        # waits on another batch's group scalars.
        alloc_small(0)
        alloc_small(1)
        phase_Ag(0, 0)
        phase_Az(0, 0)
        phase_Be(0, 0)
        phase_Ag(1, 1)
        phase_Az(1, 1)
        phase_Bs(0, 0)
        phase_Be(1, 1)
        phase_Ag(1, 0)
        phase_Az(1, 0)
        phase_Mv(0, [0, 1, 2, 3], first=True, last=False)
        phase_Bd(0, 0, True, False)
        phase_Bs(1, 1)
        phase_Be(1, 0)
        phase_Ag(0, 1)
        phase_Az(0, 1)
        phase_Mv(1, [4, 5, 6], first=True, last=False)
        phase_Bd(1, 1, True, False)
        phase_Be(0, 1, grouped=True)
        phase_Bs(1, 0)
        phase_Bd(1, 0, False, True)
        phase_Mv(1, [7], first=False, last=False)
        phase_Mv(1, [0, 1, 2, 3], first=False, last=True)
        phase_D16(1)
        phase_Bs(0, 1)
        phase_Mv(0, [4, 5, 6, 7], first=False, last=True)
        # (0,1) is the laggard half: delta on DVE (Pool is serialized on
        # the batch1 deltas right then), grouped exp shortened its chain
        phase_Bd(0, 1, False, True, eng=nc.vector)
        phase_D16(0)
        phase_M(0, [0, 1, 2, 3], first=True, last=False)
        phase_M(1, [4, 5, 6], first=True, last=False)
        phase_M(1, [7], first=False, last=False)
        phase_M(1, [0, 1, 2, 3], first=False, last=True)
        phase_Cv(1)
        phase_C(1)
        phase_D(1)
        phase_M(0, [4, 5, 6, 7], first=False, last=True)
        phase_Cv(0)
        phase_C(0)
        phase_D(0)
        phase_Dout(0)
        phase_Dout(1)

    nc.compile()
    return nc


def _get_nc(b_zero=True):
    key = ("nc", b_zero)
    if key not in _CACHE:
        _CACHE[key] = _gen_nc(b_zero)
    return _CACHE[key]


def _prep_core_inputs(grids, W, b, centroids):
    """Host-side prep: fp8/fp16 casts + per-core sharded layouts."""
    import ml_dtypes

    f8 = ml_dtypes.float8_e4m3fn
    gn = np.ascontiguousarray(
        grids.astype(f8).reshape(B, NT, 128, D).transpose(0, 2, 1, 3))
    gt = np.ascontiguousarray(
        grids.transpose(0, 2, 1)).astype(f8).reshape(B, DC, 128, N)
    wt = np.ascontiguousarray((W * 32.0).T.astype(f8)).reshape(DC, 128, K)
    bias = b.astype(np.float32).reshape(1, K)
    cent = (centroids * 64.0).astype(np.float16).reshape(K, D)
    idf = np.eye(128, dtype=np.float32)
    id16 = np.eye(128, dtype=np.float16)
    id64 = np.eye(K, dtype=np.float16)

    in_maps = []
    for c in range(N_CORES):
        sl = slice(c * LB, (c + 1) * LB)
        in_maps.append(
            {
                "gt": np.ascontiguousarray(gt[sl]),
                "gn": np.ascontiguousarray(gn[sl]),
                "wt": wt,
                "bias": bias,
                "cent": cent,
                "idf": idf,
                "id16": id16,
                "id64": id64,
            }
        )
    return in_maps


def kernel(idx, grids, W, b, centroids):
    from concourse.bass_utils import run_bass_kernel_spmd

    b_np = np.asarray(b, dtype=np.float32)
    nc = _get_nc(b_zero=not np.any(b_np))
    in_maps = _prep_core_inputs(
        np.asarray(grids, dtype=np.float32),
        np.asarray(W, dtype=np.float32),
        b_np,
        np.asarray(centroids, dtype=np.float32),
    )
    res = run_bass_kernel_spmd(nc, in_maps, core_ids=list(range(N_CORES)))
    outs = []
    for c in range(N_CORES):
        o = res.results[c]["out"]                  # [LB, 128, DC, K] f16
        o = np.transpose(o.astype(np.float32), (0, 3, 2, 1))  # [LB, K, DC, 128]
        outs.append(o.reshape(LB, K, D))
    return np.concatenate(outs, axis=0)


# revision 81
# speedup vs baseline: 1.3874x; 1.0138x over previous
"""Trainium2 Bass kernel for a NetVLAD-style VQ codebook module.

reference semantics (B=16, N=2048, D=1024, K=64):
    x = l2norm(grids, axis=D)
    logits = x @ W.T + b            # [B, N, K]
    a = softmax(logits, axis=K)
    p = einsum('bnk,bnd->bkd', a, x) - centroids * a.sum(n)
    out = l2norm(p, axis=D)

Sharding: data-parallel over batch B across 8 cores (2 batches/core).

Design (35.4us -> 26.0us vs the v2 baseline):
  - G is fed twice in fp8 (gt d-major for grams/mm1, gn n-major for mm2);
    the 8MB input stream is issued from all three DMA-capable queues
    (SP/ACT/Pool) which execute concurrently, landing by ~11us; early gt
    halves are quartered so the first gram wave starts at ~3us.  Identity
    constants arrive via DMA so Pool is free to issue from t=0.
  - Row norms s=||g||^2 via PE fp8-DoubleRow Gram tiles; diagonals
    extracted with per-tile DVE masked-STT accumulates; r = quake-rsqrt
    (fused 2-op seed, 1 Newton iter).
  - Softmax: per-tile ACT exp with per-partition scale ptr r (grouped
    DVE-prescale + one big exp for the laggard half); esum as one grouped
    DVE tensor_reduce per half; delta = (e-1)*rsc in fp8 via grouped DVE
    STTs (gpsimd cannot run TensorScalarPtr on HW).
  - mm2 in [d,k] orientation, fp8 DoubleRow pairs; rank-1 rsc correction
    (v) accumulated by separate 1-col matmuls gated only on rsc+gn, all
    sharing one PSUM bank with pac via a single 128-partition zero-region
    group; centroid correction via cent^T @ diag(-asum) closes p1.
  - Finalize: ACT Square + p116 staging, ones-matmul row norms, quake
    rn, PE rank-1 broadcast; final scaling split DVE/gpsimd-TensorTensor;
    out DMAs issued from ACT/SP/Pool queue tails.
  - Emission order hand-scheduled so the four (batch, half) softmax
    pipelines flow in DMA-arrival order and no engine queue head-of-line
    blocks another batch's critical chain.
"""

import os
import sys

sys.path.insert(0, "/opt/trn_rl_repo")

import numpy as np

import concourse.bacc as bacc
import concourse.bass as bass
import concourse.mybir as mybir
import concourse.tile as tile

B, N, D, K = 16, 2048, 1024, 64
N_CORES = 8
LB = B // N_CORES
NT = N // 128              # 16 n-tiles
DC = 8                     # 8 d-chunks
GW = 2                     # gram wave size (2 pipelines diag extracts best)
HT = NT // 2               # tiles per half

F32 = mybir.dt.float32
F16 = mybir.dt.float16
F8 = mybir.dt.float8e4
I32 = mybir.dt.int32
ALU = mybir.AluOpType
ACTF = mybir.ActivationFunctionType
PM = mybir.MatmulPerfMode
AXL = mybir.AxisListType

RSQRT_MAGIC = 0x5F3759DF

_CACHE = {}


def _emit_rsqrt(nc, pool, y, x, scratch_shape, iters=2, tag="rsq", eng=None):
    """y = x**-0.5 via quake-seed + Newton (DVE by default)."""
    e = eng if eng is not None else nc.vector
    t = pool.tile(scratch_shape, F32, tag=tag + "_t")
    u = pool.tile(scratch_shape, F32, tag=tag + "_u")
    # y = MAGIC - (x>>1) == ((x>>1) ^ -1) + (MAGIC+1), fused into 2 ops
    e.tensor_scalar(
        out=t.bitcast(I32), in0=x.bitcast(I32),
        scalar1=1, scalar2=-1, op0=ALU.arith_shift_right, op1=ALU.bitwise_xor)
    e.tensor_scalar(
        out=y.bitcast(I32), in0=t.bitcast(I32),
        scalar1=RSQRT_MAGIC + 1, scalar2=None, op0=ALU.add)
    for _ in range(iters):
        e.tensor_mul(t, y, y)
        e.tensor_mul(u, t, x)
        e.tensor_scalar(
            out=u, in0=u, scalar1=-0.5, scalar2=1.5, op0=ALU.mult, op1=ALU.add)
        e.tensor_mul(y, y, u)


def _gen_nc(b_zero=True):
    nc = bacc.Bacc(None, target_bir_lowering=False)

    gt_d = nc.dram_tensor("gt", [LB, DC, 128, N], F8, kind="ExternalInput")
    gn_d = nc.dram_tensor("gn", [LB, 128, NT, D], F8, kind="ExternalInput")
    wt_d = nc.dram_tensor("wt", [DC, 128, K], F8, kind="ExternalInput")
    bias_d = nc.dram_tensor("bias", [1, K], F32, kind="ExternalInput")
    cent_d = nc.dram_tensor("cent", [K, D], F16, kind="ExternalInput")
    idf_d = nc.dram_tensor("idf", [128, 128], F32, kind="ExternalInput")
    id16_d = nc.dram_tensor("id16", [128, 128], F16, kind="ExternalInput")
    id64_d = nc.dram_tensor("id64", [K, K], F16, kind="ExternalInput")
    out_d = nc.dram_tensor("out", [LB, 128, DC, K], F16, kind="ExternalOutput")

    from contextlib import ExitStack

    with tile.TileContext(nc) as tc, ExitStack() as ctx:
        singles = ctx.enter_context(tc.tile_pool(name="singles", bufs=1))
        gpool = ctx.enter_context(tc.tile_pool(name="gpool", bufs=2))
        work = ctx.enter_context(tc.tile_pool(name="work", bufs=2))
        scr = ctx.enter_context(tc.tile_pool(name="scr", bufs=2))
        ps_gram = ctx.enter_context(tc.tile_pool(name="ps_gram", bufs=2, space="PSUM"))
        ps_z = ctx.enter_context(tc.tile_pool(name="ps_z", bufs=1, space="PSUM"))
        ps_p1 = ctx.enter_context(tc.tile_pool(name="ps_p1", bufs=2, space="PSUM"))
        ps_small = ctx.enter_context(tc.tile_pool(name="ps_small", bufs=2, space="PSUM"))

        # ---- SBUF destination tiles ----
        wt_sb = singles.tile([128, DC, K], F8)
        bias_sb = singles.tile([1, K], F32)
        cent_sb = singles.tile([K, D], F16)
        identF = singles.tile([128, 128], F32)
        ident16 = singles.tile([128, 128], F16)
        ident64 = singles.tile([K, K], F16)

        gt_sbs = [gpool.tile([128, DC, N], F8, name=f"gt_sb{lb}", tag="gt")
                  for lb in range(LB)]
        gn_sbs = [gpool.tile([128, NT, D], F8, name=f"gn_sb{lb}", tag="gn")
                  for lb in range(LB)]

        # ---- input DMA emission (3 concurrent queues) ----
        def gt_dma(eng, lb, t0, t1):
            ns = slice(t0 * 128, t1 * 128)
            eng.dma_start(
                out=gt_sbs[lb][:, :, ns],
                in_=gt_d[lb, :, :, ns].rearrange("c p n -> p c n"))

        def gn_dma(eng, lb, t0, t1):
            eng.dma_start(
                out=gn_sbs[lb][:, t0:t1, :],
                in_=gn_d[lb, :, t0:t1, :])

        # SP queue: gt0h0 quartered for earliest gram start, then gt1h1, gn1
        gt_dma(nc.sync, 0, 0, 4)
        gt_dma(nc.sync, 0, 4, 8)
        gt_dma(nc.sync, 1, 8, 12)
        gt_dma(nc.sync, 1, 12, 16)
        gn_dma(nc.sync, 1, 0, NT // 2)           # gn1 h0 (t0-7)
        gn_dma(nc.sync, 1, NT - 2, NT)           # gn1 h1b (t14-15)
        # ACT queue (frees by ~7.5us for exp work)
        nc.scalar.dma_start(out=wt_sb, in_=wt_d.rearrange("c p k -> p c k"))
        gt_dma(nc.scalar, 0, 8, 12)
        gt_dma(nc.scalar, 0, 12, 16)
        gn_dma(nc.sync, 0, 0, NT // 2)           # gn0 h0 (SP tail; needed late)
        # Pool queue (frees by ~9us for the delta offload)
        nc.gpsimd.dma_start(out=identF, in_=idf_d[:])
        gt_dma(nc.gpsimd, 1, 0, 4)
        gt_dma(nc.gpsimd, 1, 4, 8)
        gn_dma(nc.gpsimd, 0, NT // 2, NT)        # gn0 h1
        gn_dma(nc.gpsimd, 1, NT // 2, NT - 2)    # gn1 h1a (t8-13)
        nc.gpsimd.dma_start(out=ident16, in_=id16_d[:])
        nc.gpsimd.dma_start(out=ident64, in_=id64_d[:])
        nc.gpsimd.dma_start(out=cent_sb, in_=cent_d[:])
        nc.gpsimd.dma_start(out=bias_sb, in_=bias_d[:])

        ones128 = singles.tile([128, 1], F16)
        nc.vector.memset(ones128, 1.0)
        ones64p = singles.tile([K, 128], F16)
        nc.vector.memset(ones64p, 1.0)


        if not b_zero:
            ones1p = singles.tile([1, 128], F16)
            nc.vector.memset(ones1p, 1.0)
            eb16 = singles.tile([1, K], F16)
            nc.scalar.activation(out=eb16, in_=bias_sb, func=ACTF.Exp)
            ebb_ps = ps_small.tile([128, 512], F32, tag="small", name="ebb_small")
            nc.tensor.matmul(ebb_ps[:, 0:K], ones1p[:, :], eb16[:, :],
                             start=True, stop=True)
            ebb16 = singles.tile([128, K], F16)
            nc.vector.tensor_copy(out=ebb16, in_=ebb_ps[:, 0:K])

        # ---- per-batch state ----
        st = []
        for lb in range(LB):
            keys = [
                ("s_h0", [128, HT], F32),
                ("s_h1", [128, HT], F32),
                ("r_h0", [128, HT], F32),
                ("r_h1", [128, HT], F32),
                ("e_sb", [128, NT, K], F16),
                ("dl_sb", [128, NT, K], F8),
                ("esum", [128, NT], F32),
                ("recip", [128, NT], F32),
                ("rsc16", [128, NT], F16),
                ("nrecipS", [128, NT], F16),
                ("D16", [K, K], F16),
                ("v16", [128, DC], F16),
            ]
            if not b_zero:
                keys.append(("et_sb", [128, NT, K], F16))
            st.append({
                k: work.tile(shp, dt, tag=k, name=f"{k}{lb}")
                for k, shp, dt in keys
            })

        def phase_Ag(lb, half):
            """grams + diag extract + rsqrt for one half of gt."""
            S = st[lb]
            gtp = gt_sbs[lb].rearrange("p (i two) n -> p i two n", two=2)
            s_h = S[f"s_h{half}"]
            r_h = S[f"r_h{half}"]
            for w in range(HT // GW):
                wt_ = half * (HT // GW) + w
                gps = ps_gram.tile([128, GW, 128], F32, tag="gram",
                                   name=f"gram{lb}_{wt_}")
                for ti in range(GW):
                    t = wt_ * GW + ti
                    lt = gtp[:, :, :, t * 128:(t + 1) * 128]
                    for i in range(DC // 2):
                        nc.tensor.matmul(
                            gps[:, ti, :], lt[:, i], lt[:, i],
                            start=(i == 0), stop=(i == DC // 2 - 1),
                            perf_mode=PM.DoubleRow)
                for ti in range(GW):
                    t = wt_ * GW + ti
                    th = t - half * HT
                    sc_t = scr.tile([128, 128], F32, tag="xscr")
                    nc.vector.scalar_tensor_tensor(
                        out=sc_t, in0=gps[:, ti, :], scalar=1024.0,
                        in1=identF, op0=ALU.mult, op1=ALU.mult,
                        accum_out=s_h[:, th:th + 1])
            _emit_rsqrt(nc, work, r_h, s_h, [128, HT], iters=1)

        z_count = [0]

        def phase_Az(lb, half):
            """mm1 for one half -> zps[half]."""
            S = st[lb]
            gtp = gt_sbs[lb].rearrange("p (i two) n -> p i two n", two=2)
            wtp = wt_sb.rearrange("p (i two) k -> p i two k", two=2)
            # alternate between two PSUM banks in emission order
            ztag = f"z{z_count[0] % 2}"
            z_count[0] += 1
            zps = ps_z.tile([128, HT, K], F32, tag=ztag,
                            name=f"z{lb}_{half}")
            S.setdefault("zhs", [None, None])[half] = zps
            for t in range(half * HT, (half + 1) * HT):
                for i in range(DC // 2):
                    nc.tensor.matmul(
                        zps[:, t - half * HT, :],
                        gtp[:, i, :, t * 128:(t + 1) * 128],
                        wtp[:, i], start=(i == 0), stop=(i == DC // 2 - 1),
                        perf_mode=PM.DoubleRow)

        def phase_Be(lb, half, grouped=False, accum=False):
            """exp of one half: per-tile with scale ptr (optionally with the
            ACT row-sum accumulator computing esum), or DVE-prescale + one
            grouped ACT op (shorter chain; used for the last half)."""
            S = st[lb]
            zps = S["zhs"][half]
            r_h = S[f"r_h{half}"]
            gs = slice(half * HT, (half + 1) * HT)
            if grouped:
                zs16 = work.tile([128, HT, K], F16, tag="zs16")
                nc.vector.scalar_tensor_tensor(
                    out=zs16, in0=zps[:, :, :], scalar=1.0,
                    in1=r_h.unsqueeze(2).broadcast_to((128, HT, K)),
                    op0=ALU.mult, op1=ALU.mult)
                nc.scalar.activation(
                    out=S["e_sb"][:, gs, :], in_=zs16, func=ACTF.Exp)
            else:
                for t in range(half * HT, (half + 1) * HT):
                    th = t - half * HT
                    acc = S["esum"][:, t:t + 1] if (accum and b_zero) else None
                    nc.scalar.activation(
                        out=S["e_sb"][:, t, :], in_=zps[:, th, :],
                        func=ACTF.Exp, scale=r_h[:, th:th + 1],
                        accum_out=acc)

        def phase_Bs(lb, half, esum_done=False, pooled=False):
            """esum (group reduce; optionally avg-pool on gpsimd with the
            1/64 folded into the group scalars) + group scalars."""
            S = st[lb]
            gs = slice(half * HT, (half + 1) * HT)
            rsc_scale, nrec_scale = 2.0 ** 20, -(2.0 ** 9)
            if b_zero:
                if pooled:
                    # esum/64 via gpsimd avg-pool (InstPool is in the
                    # standard gpsimd library); constants absorb the 64x.
                    # opt=False keeps the [128,HT,K] dims so the pool
                    # window stays K (the optimizer would merge them).
                    from concourse import ap_utils
                    in_pap = nc.gpsimd.lower_ap(S["e_sb"][:, gs, :],
                                                opt=False)
                    nd = len(in_pap.ap)
                    in_pap.ap = mybir.VecI64Pair(ap_utils.expand_dims_ap(
                        in_pap.ap, [i for i in range(1, 6 - nd)]))
                    nc.gpsimd.add_instruction(mybir.InstPool(
                        name=nc.get_next_instruction_name(),
                        func=mybir.PoolFunctionType.avg,
                        ins=[in_pap],
                        outs=[nc.gpsimd.lower_ap(S["esum"][:, gs])]))
                    rsc_scale, nrec_scale = 2.0 ** 14, -(2.0 ** 3)
                elif not esum_done:
                    nc.vector.tensor_reduce(
                        out=S["esum"][:, gs], in_=S["e_sb"][:, gs, :],
                        axis=AXL.X, op=ALU.add)
            else:
                for t in range(half * HT, (half + 1) * HT):
                    nc.vector.scalar_tensor_tensor(
                        out=S["et_sb"][:, t, :], in0=S["e_sb"][:, t, :],
                        scalar=1.0, in1=ebb16, op0=ALU.mult, op1=ALU.mult,
                        accum_out=S["esum"][:, t:t + 1])
            nc.vector.reciprocal(out=S["recip"][:, gs], in_=S["esum"][:, gs])
            # rsc = recip * r * scale (one fused STT)
            nc.vector.scalar_tensor_tensor(
                out=S["rsc16"][:, gs], in0=S["recip"][:, gs],
                scalar=rsc_scale, in1=S[f"r_h{half}"],
                op0=ALU.mult, op1=ALU.mult)
            nc.vector.tensor_scalar_mul(S["nrecipS"][:, gs],
                                        S["recip"][:, gs], nrec_scale)

        def phase_Bd(lb, half, pac_first, pac_last, eng=None):
            """delta (group STT, gpsimd by default) + pac matmuls."""
            S = st[lb]
            src = S["e_sb"] if b_zero else S["et_sb"]
            gs = slice(half * HT, (half + 1) * HT)
            rb = S["rsc16"][:, gs].unsqueeze(2).broadcast_to((128, HT, K))
            # gpsimd can only run TensorTensor on HW, so deltas run on DVE
            # (grouped STT) or ACT (per-tile Identity with scale/bias ptrs)
            if eng is nc.scalar:
                nrsc = work.tile([128, HT], F32, tag="nrsc")
                nc.scalar.activation(out=nrsc, in_=S["rsc32"][:, gs],
                                     func=ACTF.Identity, scale=-1.0)
                for t in range(half * HT, (half + 1) * HT):
                    th = t - half * HT
                    nc.scalar.activation(
                        out=S["dl_sb"][:, t, :], in_=src[:, t, :],
                        func=ACTF.Identity,
                        scale=S["rsc32"][:, t:t + 1],
                        bias=nrsc[:, th:th + 1])
            else:
                deng = eng if eng is not None else nc.vector
                deng.scalar_tensor_tensor(
                    out=S["dl_sb"][:, gs, :], in0=src[:, gs, :],
                    scalar=-1.0, in1=rb, op0=ALU.add, op1=ALU.mult)
            ts = list(range(half * HT, (half + 1) * HT))
            for t in ts:
                nc.tensor.matmul(
                    S["pac"], src[:, t, :], S["nrecipS"][:, t:t + 1],
                    start=False, stop=False, skip_group_check=True)

        def alloc_small(lb):
            S = st[lb]
            small = ps_small.tile([128, 512], F32, tag="small",
                                  name=f"small{lb}")
            S["small"] = small
            S["pac"] = small[0:K, 0:1]

        def phase_D16(lb):
            S = st[lb]
            nc.vector.tensor_scalar(
                out=S["D16"], in0=ident64, scalar1=S["pac"], scalar2=None,
                op0=ALU.mult)

        def phase_Mv(lb, prs, first, last):
            """v accumulation for gn pair-rows (needs only rsc + gn).

            The v group's start (a 128-partition op) zero-marks the whole
            PSUM bank; the 64-partition pac matmuls ride along with
            skip_group_check and get lazily zeroed on first write."""
            S = st[lb]
            v_ps = S["small"][:, 8:8 + DC]
            gn_sb = gn_sbs[lb]
            for j, pr in enumerate(prs):
                for k2, t in enumerate((2 * pr, 2 * pr + 1)):
                    for c in range(DC):
                        nc.tensor.matmul(
                            v_ps[:, c:c + 1],
                            gn_sb[:, t, c * 128:(c + 1) * 128],
                            S["rsc16"][:, t:t + 1],
                            start=(first and j == 0 and k2 == 0 and c == 0),
                            stop=(last and j == len(prs) - 1 and k2 == 1
                                  and c == DC - 1))

        def phase_M(lb, prs, first, last):
            """mm2 (fp8 DR) for gn pair-rows `prs`."""
            S = st[lb]
            if first:
                p1 = ps_p1.tile([128, DC, K], F32, tag="p1", name=f"p1_{lb}")
                S["p1"] = p1
            p1 = S["p1"]
            dlp = S["dl_sb"].rearrange("p (pr two) k -> p pr two k", two=2)
            gnp = gn_sbs[lb].rearrange("p (pr two) d -> p pr two d", two=2)
            for j, pr in enumerate(prs):
                for c in range(DC):
                    nc.tensor.matmul(
                        p1[:, c, :],
                        gnp[:, pr, :, c * 128:(c + 1) * 128],
                        dlp[:, pr],
                        start=(first and j == 0 and c == 0),
                        stop=False,
                        perf_mode=PM.DoubleRow)

        def phase_Cv(lb):
            S = st[lb]
            nc.vector.tensor_copy(out=S["v16"], in_=S["small"][:, 8:8 + DC])

        def phase_C(lb):
            """close p1: + v broadcast + cent correction."""
            S = st[lb]
            p1 = S["p1"]
            for c in range(DC):
                nc.tensor.matmul(
                    p1[:, c, :], ident16[:, :],
                    S["v16"][:, c:c + 1].broadcast_to((128, K)),
                    start=False, stop=False)
            for c in range(DC):
                nc.tensor.matmul(
                    p1[:, c, :], cent_sb[:, c * 128:(c + 1) * 128], S["D16"],
                    start=False, stop=(c == DC - 1))

        def phase_D(lb):
            """finalize: qq (ACT||DVE halves), n2, rn, rnb, scale, out DMA."""
            S = st[lb]
            small = S["small"]
            p1 = S["p1"]
            n2_ps = small[0:K, 16 + DC:16 + DC + 1]
            rnb_ps = small[:, 128:128 + K]
            qq16 = work.tile([128, DC, K], F16, tag="qq")
            p116 = work.tile([128, DC, K], F16, tag="p116")
            hc = DC // 2
            h0 = slice(0, hc)
            h1 = slice(hc, DC)
            # ACT squares the whole tile, then stages p116 while PE sums n2
            nc.scalar.activation(out=qq16, in_=p1,
                                 func=ACTF.Square, scale=2.0 ** -10)
            nc.scalar.activation(out=p116, in_=p1,
                                 func=ACTF.Identity, scale=2.0 ** -10)
            for c in range(DC):
                nc.tensor.matmul(
                    n2_ps, qq16[:, c, :], ones128[:, 0:1],
                    start=(c == 0), stop=(c == DC - 1))
            n2_sb = work.tile([K, 1], F32, tag="n2sb")
            nc.vector.tensor_copy(out=n2_sb, in_=n2_ps)
            rn_sb = work.tile([K, 1], F32, tag="rn")
            rne = nc.vector
            _emit_rsqrt(nc, work, rn_sb, n2_sb, [K, 1], iters=1, tag="rsq2",
                        eng=rne)
            rnD = work.tile([K, K], F16, tag="rnD")
            rne.tensor_scalar(
                out=rnD, in0=ident64, scalar1=rn_sb, scalar2=None, op0=ALU.mult)
            nc.tensor.matmul(rnb_ps, ones64p[:, :], rnD[:, :],
                             start=True, stop=True)
            rnb16 = work.tile([128, K], F16, tag="rnb16")
            nc.vector.tensor_copy(out=rnb16, in_=rnb_ps)
            out_sb = work.tile([128, DC, K], F16, tag="out_sb")
            S["out_sb"] = out_sb
            if lb == 0:
                # tail batch: final scaling halves in parallel DVE || gpsimd
                # (TensorTensor — the only elementwise op gpsimd runs on HW)
                nc.vector.tensor_tensor(
                    out=out_sb[:, h0, :], in0=p116[:, h0, :],
                    in1=rnb16.unsqueeze(1).broadcast_to((128, hc, K)),
                    op=ALU.mult)
                nc.gpsimd.tensor_tensor(
                    out=out_sb[:, h1, :], in0=p116[:, h1, :],
                    in1=rnb16.unsqueeze(1).broadcast_to((128, hc, K)),
                    op=ALU.mult)
            else:
                # keep DVE free for the tail batch's rn chain
                nc.gpsimd.tensor_tensor(
                    out=out_sb, in0=p116,
                    in1=rnb16.unsqueeze(1).broadcast_to((128, DC, K)),
                    op=ALU.mult)

        def phase_Dout(lb):
            S = st[lb]
            out_sb = S["out_sb"]
            hc = DC // 2
            if lb == 0:
                # batch0 finishes last -> fast split out path
                nc.scalar.dma_start(out=out_d[lb, :, 0:hc],
                                    in_=out_sb[:, 0:hc, :])
                nc.sync.dma_start(out=out_d[lb, :, hc:DC],
                                  in_=out_sb[:, hc:DC, :])
            else:
                nc.gpsimd.dma_start(out=out_d[lb, :, :], in_=out_sb[:, :, :])

        # ---- schedule (emission order == per-engine queue order) ----
        # Half pipelines flow in order (0,0), (1,1), (1,0), (0,1) matching
        # DMA arrival; diag/rsqrt prioritized on DVE so no half's softmax
        # waits on another batch's group scalars.
        alloc_small(0)
        alloc_small(1)
        phase_Ag(0, 0)
        phase_Az(0, 0)
        phase_Be(0, 0)
        phase_Ag(1, 1)
        phase_Az(1, 1)
        phase_Bs(0, 0)
        phase_Be(1, 1)
        phase_Ag(1, 0)
        phase_Az(1, 0)
        phase_Mv(0, [0, 1, 2, 3], first=True, last=False)
        phase_Bd(0, 0, True, False)
        phase_Bs(1, 1)
        phase_Be(1, 0)
        phase_Ag(0, 1)
        phase_Az(0, 1)
        phase_Mv(1, [4, 5, 6], first=True, last=False)
        phase_Bd(1, 1, True, False)
        phase_Be(0, 1, grouped=True)
        phase_Bs(1, 0)
        phase_Bd(1, 0, False, True)
        phase_Mv(1, [7], first=False, last=False)
        phase_Mv(1, [0, 1, 2, 3], first=False, last=True)
        phase_D16(1)
        phase_Bs(0, 1)
        phase_Mv(0, [4, 5, 6, 7], first=False, last=True)
        # (0,1) is the laggard half: delta on DVE (Pool is serialized on
        # the batch1 deltas right then), grouped exp shortened its chain
        phase_Bd(0, 1, False, True, eng=nc.vector)
        phase_D16(0)
        phase_M(0, [0, 1, 2, 3], first=True, last=False)
        phase_M(1, [4, 5, 6], first=True, last=False)
        phase_M(1, [7], first=False, last=False)
        phase_M(1, [0, 1, 2, 3], first=False, last=True)
        phase_Cv(1)
        phase_C(1)
        phase_D(1)
        phase_M(0, [4, 5, 6, 7], first=False, last=True)
        phase_Cv(0)
        phase_C(0)
        phase_D(0)
        phase_Dout(0)
        phase_Dout(1)

    nc.compile()
    return nc


def _get_nc(b_zero=True):
    key = ("nc", b_zero)
    if key not in _CACHE:
        _CACHE[key] = _gen_nc(b_zero)
    return _CACHE[key]


def _prep_core_inputs(grids, W, b, centroids):
    """Host-side prep: fp8/fp16 casts + per-core sharded layouts."""
    import ml_dtypes

    f8 = ml_dtypes.float8_e4m3fn
    gn = np.ascontiguousarray(
        grids.astype(f8).reshape(B, NT, 128, D).transpose(0, 2, 1, 3))
    gt = np.ascontiguousarray(
        grids.transpose(0, 2, 1)).astype(f8).reshape(B, DC, 128, N)
    wt = np.ascontiguousarray((W * 32.0).T.astype(f8)).reshape(DC, 128, K)
    bias = b.astype(np.float32).reshape(1, K)
    cent = (centroids * 64.0).astype(np.float16).reshape(K, D)
    idf = np.eye(128, dtype=np.float32)
    id16 = np.eye(128, dtype=np.float16)
    id64 = np.eye(K, dtype=np.float16)

    in_maps = []
    for c in range(N_CORES):
        sl = slice(c * LB, (c + 1) * LB)
        in_maps.append(
            {
                "gt": np.ascontiguousarray(gt[sl]),
                "gn": np.ascontiguousarray(gn[sl]),
                "wt": wt,
                "bias": bias,
                "cent": cent,
                "idf": idf,
                "id16": id16,
                "id64": id64,
            }
        )
    return in_maps


def kernel(idx, grids, W, b, centroids):
    from concourse.bass_utils import run_bass_kernel_spmd

    b_np = np.asarray(b, dtype=np.float32)
    nc = _get_nc(b_zero=not np.any(b_np))
    in_maps = _prep_core_inputs(
        np.asarray(grids, dtype=np.float32),
        np.asarray(W, dtype=np.float32),
        b_np,
        np.asarray(centroids, dtype=np.float32),
    )
    res = run_bass_kernel_spmd(nc, in_maps, core_ids=list(range(N_CORES)))
    outs = []
    for c in range(N_CORES):
        o = res.results[c]["out"]                  # [LB, 128, DC, K] f16
        o = np.transpose(o.astype(np.float32), (0, 3, 2, 1))  # [LB, K, DC, 128]
        outs.append(o.reshape(LB, K, D))
    return np.concatenate(outs, axis=0)
